# revision 19
# baseline (speedup 1.0000x reference)
"""V5: full-bf16, fully SBUF-resident rewrite of the V3 baseline.

Design (8-core tensor-parallel over heads, no device collectives; each
core computes a partial y over its 2 heads, host sums partials in f64):

- All matmuls bf16 (1 cyc/col on PE, same rate as f32r but half the DMA/
  SBUF, FWL weight loads, no f32r 4x penalty on narrow diagonal tiles).
- q/k/v/attention-out never leave SBUF (V3 spilled q, v, half of o to
  DRAM: ~17MB/core of round-trip DMA eliminated). y written bf16.
- Host pre-packs x and Wqkv into the exact SBUF tile layout so every
  load is 128 contiguous 16-24KB runs per partition.
- PSUM (8 banks, one open accumulation group per bank): qk-proj, y-proj
  and softmax-denominator share a 3-buf ring; v-proj and attention-out
  share a 2-buf ring; score tiles triple-buffered.
- Attention + y-projection for each 512-token slice are emitted right
  after the NEXT phase-1 block (blocks are qs-sized), so exp work
  spreads into phase 1's idle ACT time instead of bunching at the end.
- Causal mask applied AFTER exp as a 0/1 bf16 multiply (DVE 4x mode)
  so each k-tile needs a single ACT exp call.
- Softmax denominator: exp tiles are pre-summed on DVE in bf16 (4x
  mode; unmasked tiles and diagonal tiles in separate accumulators) and
  reduced with a single pair of ones-matmuls per (bh, qs), so the pd
  PSUM bank is held for ~400ns instead of a whole iteration.
- RoPE pipeline fully bf16 (cos/sin tables, temporaries) for DVE 4x.

TimelineSim 325us/core (PE 283us busy, 87%); HW (For_i-loop slope)
~410-426us vs the V3 baseline's 487us in same-process A/B; rel err vs
the f32 reference 7.7e-3.
"""

import math
from contextlib import ExitStack

import numpy as np
import ml_dtypes

import concourse.bass as bass
import concourse.tile as tile
from concourse import bacc, mybir
from concourse.bass_utils import run_bass_kernel_spmd

B, L, H, NH, HD = 2, 2048, 2048, 16, 128
ROPE_THETA = 10000.0
N_CORES = 8
NH_LOC = NH // N_CORES          # 2
QKV_LOC = 3 * NH_LOC * HD       # 768
D_LOC = NH_LOC * HD             # 256
BL = B * L
P = 128
KC = H // P                     # 16
BLK = 512
NBLK = BL // BLK                # 8
BLK_PER_B = NBLK // B           # 4
QS = 512
NQS = L // QS
KT = L // P
NBH = B * NH_LOC                # 4

F32 = mybir.dt.float32
BF16 = mybir.dt.bfloat16
EXP = mybir.ActivationFunctionType.Exp
NEG = -30000.0


def _build(loop=None):
    nc = bacc.Bacc("TRN2", target_bir_lowering=False, debug=False,
                   num_devices=N_CORES)

    xI = nc.dram_tensor("xI", [P, NBLK, KC, BLK], BF16,
                        kind="ExternalInput").ap()
    wI = nc.dram_tensor("wI", [P, KC, QKV_LOC], BF16,
                        kind="ExternalInput").ap()
    woT = nc.dram_tensor("woT", [D_LOC, H], BF16, kind="ExternalInput").ap()
    cosT = nc.dram_tensor("cosT", [HD, L], BF16, kind="ExternalInput").ap()
    sinTs = nc.dram_tensor("sinTs", [HD, L], BF16, kind="ExternalInput").ap()
    tri = nc.dram_tensor("tri", [P, P], BF16, kind="ExternalInput").ap()
    ones_in = nc.dram_tensor("ones", [P, P], BF16, kind="ExternalInput").ap()
    y = nc.dram_tensor("y", [BL, H], BF16, kind="ExternalOutput").ap()

    with tile.TileContext(nc) as tc, ExitStack() as ctx:
        if loop:
            ctx.enter_context(tc.For_i(0, loop, 1))
        g = ctx.enter_context(tc.tile_pool(name="g", bufs=1))
        wt = g.tile([P, KC, QKV_LOC], BF16)
        cost = g.tile([P, L], BF16)
        sints = g.tile([P, L], BF16)
        k_all = g.tile([P, NBH, L], BF16)
        q_all = g.tile([P, NBH, L], BF16)
        v_all = g.tile([P, NBH, KT, HD], BF16)
        o_all = g.tile([P, B, NH_LOC, L], BF16)
        wo = g.tile([P, NH_LOC, H], BF16)
        trimask = g.tile([P, P], BF16)
        ones = g.tile([P, P], BF16)

        p1x = ctx.enter_context(tc.tile_pool(name="p1x", bufs=2))
        p1q = ctx.enter_context(tc.tile_pool(name="p1q", bufs=2))
        p1t = ctx.enter_context(tc.tile_pool(name="p1t", bufs=4))
        p2e = ctx.enter_context(tc.tile_pool(name="p2e", bufs=8))
        p2t = ctx.enter_context(tc.tile_pool(name="p2t", bufs=6))
        p3y = ctx.enter_context(tc.tile_pool(name="p3y", bufs=3))
        # PSUM: bank-granular. psA: phase-1 qk pairs + phase-3 y groups;
        # psVO: phase-1 v + phase-2 po; psS: score tiles; psD: denominators.
        psA = ctx.enter_context(tc.tile_pool(name="psA", bufs=3, space="PSUM"))
        psVO = ctx.enter_context(tc.tile_pool(name="psVO", bufs=2, space="PSUM"))
        psS = ctx.enter_context(tc.tile_pool(name="psS", bufs=3, space="PSUM"))

        # ------- weight / x(block0) loads, interleaved in kc order -------
        xb0 = p1x.tile([P, KC, BLK], BF16, name="xb")
        for c0_, c1_ in ((0, 1), (1, 2), (2, 4), (4, 8), (8, 12), (12, 16)):
            nc.sync.dma_start(wt[:, c0_:c1_, :], wI[:, c0_:c1_, :])
            nc.sync.dma_start(xb0[:, c0_:c1_, :], xI[:, 0, c0_:c1_, :])
        xb1 = p1x.tile([P, KC, BLK], BF16, name="xb")
        for half in range(2):
            nc.sync.dma_start(
                xb1[:, half * 8:(half + 1) * 8, :],
                xI[:, 1, half * 8:(half + 1) * 8, :])
        for ch in range(4):
            sl = slice(ch * 512, (ch + 1) * 512)
            nc.sync.dma_start(cost[:, sl], cosT[:, sl])
            nc.sync.dma_start(sints[:, sl], sinTs[:, sl])
        nc.sync.dma_start(trimask[:], tri[:])
        nc.sync.dma_start(ones[:], ones_in[:])
        for hh in range(NH_LOC):
            nc.sync.dma_start(wo[:, hh, :], woT[hh * P:(hh + 1) * P, :])

        copy_flip = [0]

        def emit_att(b, qs_i):
            qs = qs_i * QS
            nkt = (qs + QS) // P
            for hh in range(NH_LOC):
                bh = b * NH_LOC + hh
                po = psVO.tile([P, QS], F32, name="vo")
                full = qs // P      # k-tiles with no masking this qs
                if full > 0:
                    esum = p2t.tile([P, QS], BF16, name="esum")
                esum2 = p2t.tile([P, QS], BF16, name="esum")

                # 2-stage software pipeline: emit the score-MM + exp for
                # k+2 before the AV-MM of k, so the exp has ~2 matmuls of
                # latency to complete before the in-order PE needs et(k).
                ets = {}

                def stage1(k_i):
                    d = k_i * P - qs
                    c0 = max(d, 0)
                    psc = psS.tile([P, QS], F32, name="psc")
                    nc.tensor.matmul(
                        psc[:, c0:QS],
                        lhsT=k_all[:, bh, k_i * P:(k_i + 1) * P],
                        rhs=q_all[:, bh, qs + c0:qs + QS],
                        start=True, stop=True)
                    et = p2e.tile([P, QS], BF16, name="et")
                    nc.scalar.activation(et[:, c0:QS], psc[:, c0:QS], EXP)
                    ets[k_i] = et

                def stage2(k_i):
                    d = k_i * P - qs
                    c0 = max(d, 0)
                    et = ets.pop(k_i)
                    if d >= 0:
                        nc.vector.tensor_mul(et[:, d:d + P],
                                             et[:, d:d + P], trimask[:])
                    nc.tensor.matmul(po[:, c0:QS],
                                     lhsT=v_all[:, bh, k_i, :],
                                     rhs=et[:, c0:QS], start=(k_i == 0),
                                     stop=(k_i == nkt - 1))
                    # denominator partials in bf16 on DVE (4x): unmasked
                    # tiles into esum, diagonal ones into esum2, so the pd
                    # PSUM bank is only held for the final MM pair
                    if k_i < full:
                        if k_i == 0:
                            nc.vector.tensor_copy(esum[:], et[:])
                        else:
                            nc.vector.tensor_add(esum[:], esum[:], et[:])
                    elif k_i == full:
                        nc.vector.tensor_copy(esum2[:, c0:QS], et[:, c0:QS])
                    else:
                        nc.vector.tensor_add(esum2[:, c0:QS],
                                             esum2[:, c0:QS], et[:, c0:QS])

                for k_i in range(min(3, nkt)):
                    stage1(k_i)
                for k_i in range(nkt):
                    if k_i + 3 < nkt:
                        stage1(k_i + 3)
                    stage2(k_i)
                pd = psA.tile([P, QS], F32, name="acc")
                nc.tensor.matmul(pd[:], lhsT=ones[:], rhs=esum2[:],
                                 start=True, stop=(full == 0))
                if full > 0:
                    nc.tensor.matmul(pd[:], lhsT=ones[:], rhs=esum[:],
                                     start=False, stop=True)
                rec = p2t.tile([P, QS], F32, name="rec")
                if (b, qs_i) == (1, NQS - 1):
                    # split so the trailing y-projection can pipeline
                    for tq in range(4):
                        s_ = slice(tq * P, (tq + 1) * P)
                        nc.vector.reciprocal(rec[:, s_], pd[:, s_])
                        nc.vector.tensor_mul(
                            o_all[:, b, hh, qs + tq * P:qs + (tq + 1) * P],
                            po[:, s_], rec[:, s_])
                else:
                    nc.vector.reciprocal(rec[:], pd[:])
                    nc.vector.tensor_mul(o_all[:, b, hh, qs:qs + QS],
                                         po[:], rec[:])

        def emit_p3(b, qs_i):
            qs = qs_i * QS
            for tt in range(qs // P, (qs + QS) // P):
                ybig = p3y.tile([P, H], BF16, name="ybig")
                for oc in range(H // QS):
                    py_ = psA.tile([P, QS], F32, name="acc")
                    for hh in range(NH_LOC):
                        nc.tensor.matmul(
                            py_[:],
                            lhsT=o_all[:, b, hh, tt * P:(tt + 1) * P],
                            rhs=wo[:, hh, oc * QS:(oc + 1) * QS],
                            start=(hh == 0), stop=(hh == NH_LOC - 1))
                    if copy_flip[0] % 2 == 0:
                        nc.vector.tensor_copy(
                            ybig[:, oc * QS:(oc + 1) * QS], py_[:])
                    else:
                        nc.scalar.copy(
                            ybig[:, oc * QS:(oc + 1) * QS], py_[:])
                    copy_flip[0] += 1
                rows = slice(b * L + tt * P, b * L + (tt + 1) * P)
                nc.sync.dma_start(y[rows, 0:H // 2], ybig[:, 0:H // 2])
                nc.sync.dma_start(y[rows, H // 2:H], ybig[:, H // 2:H])

        # ---------------- phase 1: QKV projection + RoPE ----------------
        for blk in range(NBLK):
            b, lo = divmod(blk, BLK_PER_B)
            lo *= BLK
            col = blk * BLK
            if blk == 0:
                xb = xb0
            elif blk == 1:
                xb = xb1
            else:
                xb = p1x.tile([P, KC, BLK], BF16, name="xb")
                nc.sync.dma_start(xb[:], xI[:, blk, :, :])

            qc = p1q.tile([P, 4, BLK], BF16, name="qc")
            for dt_i in range(4):     # 0,1 = q heads; 2,3 = k heads
                psum = psA.tile([P, QS], F32, name="acc")
                for kc in range(KC):
                    nc.tensor.matmul(
                        psum[:], lhsT=wt[:, kc, dt_i * P:dt_i * P + P],
                        rhs=xb[:, kc, :],
                        start=(kc == 0), stop=(kc == KC - 1))
                nc.scalar.copy(qc[:, dt_i, :], psum[:])
            # batched rotate-half swap for all 4 dim-tiles
            qsw = p1q.tile([P, 4, BLK], BF16, name="qsw")
            nc.sync.dma_start(qsw[0:64, :, :], qc[64:128, :, :])
            nc.sync.dma_start(qsw[64:128, :, :], qc[0:64, :, :])

            for dt_i in range(4):
                qk, hh = divmod(dt_i, 2)
                bh = b * NH_LOC + hh
                t1 = p1t.tile([P, BLK], BF16, name="t1")
                nc.vector.tensor_mul(t1[:], qc[:, dt_i, :],
                                     cost[:, lo:lo + BLK])
                t2 = p1t.tile([P, BLK], BF16, name="t2")
                nc.vector.tensor_mul(t2[:], qsw[:, dt_i, :],
                                     sints[:, lo:lo + BLK])
                if qk == 0:
                    nc.vector.tensor_add(q_all[:, bh, lo:lo + BLK],
                                         t1[:], t2[:])
                else:
                    nc.vector.tensor_add(k_all[:, bh, lo:lo + BLK],
                                         t1[:], t2[:])

            for tp in range(2):
                psv = psVO.tile([P, QS], F32, name="vo")
                for tj in range(2):
                    tt = tp * 2 + tj
                    for kc in range(KC):
                        nc.tensor.matmul(
                            psv[:, tj * D_LOC:(tj + 1) * D_LOC],
                            lhsT=xb[:, kc, tt * P:(tt + 1) * P],
                            rhs=wt[:, kc, 2 * D_LOC:3 * D_LOC],
                            start=(kc == 0), stop=(kc == KC - 1))
                    kt = lo // P + tt
                    for hh in range(NH_LOC):
                        nc.scalar.copy(
                            v_all[:, b * NH_LOC + hh, kt, :],
                            psv[:, tj * D_LOC + hh * HD:
                                tj * D_LOC + (hh + 1) * HD])

            if blk > 0:
                pb, pq = divmod(blk - 1, BLK_PER_B)
                emit_att(pb, pq)
                emit_p3(pb, pq)

        # ------- final slice: attention + y-projection for (b1, qs3) ----
        emit_att(1, NQS - 1)
        emit_p3(1, NQS - 1)

           full = qs // P      # k-tiles with no masking this qs
                    if full > 0:
                        esum = p2t.tile([P, QS], BF16, name="esum")
                    esum2 = p2t.tile([P, QS], BF16, name="esum")
                    for k_i in range(nkt):
                        d = k_i * P - qs
                        c0 = max(d, 0)
                        psc = psS.tile([P, QS], F32, name="psc")
                        nc.tensor.matmul(
                            psc[:, c0:QS],
                            lhsT=k_all[:, bh, k_i * P:(k_i + 1) * P],
                            rhs=q_all[:, bh, qs + c0:qs + QS],
                            start=True, stop=True)
                        et = p2e.tile([P, QS], BF16, name="et")
                        nc.scalar.activation(et[:, c0:QS], psc[:, c0:QS], EXP)
                        if d >= 0:
                            nc.vector.tensor_mul(et[:, d:d + P],
                                                 et[:, d:d + P], trimask[:])
                        nc.tensor.matmul(po[:, c0:QS],
                                         lhsT=v_all[:, bh, k_i, :],
                                         rhs=et[:, c0:QS], start=(k_i == 0),
                                         stop=(k_i == nkt - 1))
                        # denominator partials in bf16 on DVE (4x mode):
                        # unmasked tiles into esum, diagonal ones into esum2,
                        # so the pd PSUM bank is only held for the final MMs
                        if k_i < full:
                            if k_i == 0:
                                nc.vector.tensor_copy(esum[:], et[:])
                            else:
                                nc.vector.tensor_add(esum[:], esum[:], et[:])
                        elif k_i == full:
                            nc.vector.tensor_copy(esum2[:, c0:QS],
                                                  et[:, c0:QS])
                        else:
                            nc.vector.tensor_add(esum2[:, c0:QS],
                                                 esum2[:, c0:QS],
                                                 et[:, c0:QS])
                    pd = psA.tile([P, QS], F32, name="acc")
                    nc.tensor.matmul(pd[:], lhsT=ones[:], rhs=esum2[:],
                                     start=True, stop=(full == 0))
                    if full > 0:
                        nc.tensor.matmul(pd[:], lhsT=ones[:], rhs=esum[:],
                                         start=False, stop=True)
                    rec = p2t.tile([P, QS], F32, name="rec")
                    if qs_i == NQS - 1:
                        # split so the trailing y-projection can pipeline
                        for tq in range(4):
                            s_ = slice(tq * P, (tq + 1) * P)
                            nc.vector.reciprocal(rec[:, s_], pd[:, s_])
                            nc.vector.tensor_mul(
                                o_all[:, b, hh, qs + tq * P:qs + (tq + 1) * P],
                                po[:, s_], rec[:, s_])
                    else:
                        nc.vector.reciprocal(rec[:], pd[:])
                        nc.vector.tensor_mul(o_all[:, b, hh, qs:qs + QS],
                                             po[:], rec[:])
                # y-projection for this batch's freshly finished qs tokens
                for tt in range(qs // P, (qs + QS) // P):
                    ybig = p3y.tile([P, H], BF16, name="ybig")
                    for oc in range(H // QS):
                        py_ = psA.tile([P, QS], F32, name="acc")
                        for hh in range(NH_LOC):
                            nc.tensor.matmul(
                                py_[:],
                                lhsT=o_all[:, b, hh, tt * P:(tt + 1) * P],
                                rhs=wo[:, hh, oc * QS:(oc + 1) * QS],
                                start=(hh == 0), stop=(hh == NH_LOC - 1))
                        if copy_flip % 2 == 0:
                            nc.vector.tensor_copy(
                                ybig[:, oc * QS:(oc + 1) * QS], py_[:])
                        else:
                            nc.scalar.copy(
                                ybig[:, oc * QS:(oc + 1) * QS], py_[:])
                        copy_flip += 1
                    rows = slice(b * L + tt * P, b * L + (tt + 1) * P)
                    nc.sync.dma_start(y[rows, 0:H // 2], ybig[:, 0:H // 2])
                    nc.sync.dma_start(y[rows, H // 2:H], ybig[:, H // 2:H])

    nc.compile()
    return nc


_NC = None


def _get_nc():
    global _NC
    if _NC is None:
        _NC = _build()
    return _NC


def _host_inputs(x, Wqkv, Wo):
    x = np.asarray(x, dtype=np.float32)
    Wqkv = np.asarray(Wqkv, dtype=np.float32)
    Wo = np.asarray(Wo, dtype=np.float32)

    xTb = x.reshape(BL, H).T.astype(ml_dtypes.bfloat16)   # (H, BL)
    xI = np.ascontiguousarray(
        xTb.reshape(KC, P, NBLK, BLK).transpose(1, 2, 0, 3))

    inv_freq = 1.0 / (ROPE_THETA ** (np.arange(0, HD, 2, dtype=np.float32)
                                     / HD))
    t = np.arange(L, dtype=np.float32)
    freqs = np.outer(t, inv_freq).astype(np.float32)
    emb = np.concatenate([freqs, freqs], axis=-1)
    cosT = np.ascontiguousarray(np.cos(emb).T.astype(ml_dtypes.bfloat16))
    sinT = np.sin(emb).T.astype(np.float32)
    sinTs = np.ascontiguousarray(
        np.concatenate([-sinT[:64], sinT[64:]], 0).astype(ml_dtypes.bfloat16))

    kk = np.arange(P)[:, None]
    qq = np.arange(P)[None, :]
    tri = np.where(qq >= kk, 1.0, 0.0).astype(ml_dtypes.bfloat16)

    scale = np.float32(1.0 / math.sqrt(HD))
    in_maps = []
    for c in range(N_CORES):
        r0 = c * D_LOC
        wq = Wqkv[r0:r0 + D_LOC] * scale
        wk = Wqkv[H + r0:H + r0 + D_LOC]
        wv = Wqkv[2 * H + r0:2 * H + r0 + D_LOC]
        wT_c = np.concatenate([wq, wk, wv], 0).T.astype(ml_dtypes.bfloat16)
        wI_c = np.ascontiguousarray(
            wT_c.reshape(KC, P, QKV_LOC).transpose(1, 0, 2))
        woT_c = np.ascontiguousarray(
            Wo[:, r0:r0 + D_LOC].T.astype(ml_dtypes.bfloat16))
        in_maps.append({
            "xI": xI, "wI": wI_c, "woT": woT_c,
            "cosT": cosT, "sinTs": sinTs, "tri": tri,
            "ones": np.ones((P, P), dtype=ml_dtypes.bfloat16),
        })
    return in_maps


def kernel(x, Wqkv, Wo):
    nc = _get_nc()
    in_maps = _host_inputs(x, Wqkv, Wo)
    res = run_bass_kernel_spmd(nc, in_maps, list(range(N_CORES)))
    y = res.results[0]["y"].astype(np.float64)
    for c in range(1, N_CORES):
        y += res.results[c]["y"].astype(np.float64)
    return y.astype(np.float32).reshape(B, L, H)


# revision 20
# speedup vs baseline: 1.0149x; 1.0149x over previous
"""V5: full-bf16, fully SBUF-resident rewrite of the V3 baseline.

Design (8-core tensor-parallel over heads, no device collectives; each
core computes a partial y over its 2 heads, host sums partials in f64):

- All matmuls bf16 (1 cyc/col on PE, same rate as f32r but half the DMA/
  SBUF, FWL weight loads, no f32r 4x penalty on narrow diagonal tiles).
- q/k/v/attention-out never leave SBUF (V3 spilled q, v, half of o to
  DRAM: ~17MB/core of round-trip DMA eliminated). y written bf16.
- Host pre-packs x and Wqkv into the exact SBUF tile layout so every
  load is 128 contiguous 16-24KB runs per partition.
- PSUM (8 banks, one open accumulation group per bank): qk-proj, y-proj
  and softmax-denominator share a 3-buf ring; v-proj and attention-out
  share a 2-buf ring; score tiles triple-buffered.
- Attention + y-projection for each 512-token slice are emitted right
  after the NEXT phase-1 block (blocks are qs-sized), so exp work
  spreads into phase 1's idle ACT time instead of bunching at the end.
- Causal mask applied AFTER exp as a 0/1 bf16 multiply (DVE 4x mode)
  so each k-tile needs a single ACT exp call.
- Attention k-loop is software-pipelined 3 deep (score-MM + exp for
  k+3 emitted before the AV-MM of k), so the in-order PE never waits
  on an exp: the ~700ns exp latency is covered by three matmuls.
- Softmax denominator: exp tiles are pre-summed on DVE in bf16 (4x
  mode; unmasked tiles and diagonal tiles in separate accumulators) and
  reduced with a single pair of ones-matmuls per (bh, qs), so the pd
  PSUM bank is held for ~400ns instead of a whole iteration.
- RoPE pipeline fully bf16 (cos/sin tables, temporaries) for DVE 4x.

TimelineSim 320us/core (PE 283us busy, 88%); HW (For_i-loop slope)
~403-419us vs the V3 baseline's ~485-499us in same-process A/B; rel
err vs the f32 reference 7.7e-3.
"""

import math
from contextlib import ExitStack

import numpy as np
import ml_dtypes

import concourse.bass as bass
import concourse.tile as tile
from concourse import bacc, mybir
from concourse.bass_utils import run_bass_kernel_spmd

B, L, H, NH, HD = 2, 2048, 2048, 16, 128
ROPE_THETA = 10000.0
N_CORES = 8
NH_LOC = NH // N_CORES          # 2
QKV_LOC = 3 * NH_LOC * HD       # 768
D_LOC = NH_LOC * HD             # 256
BL = B * L
P = 128
KC = H // P                     # 16
BLK = 512
NBLK = BL // BLK                # 8
BLK_PER_B = NBLK // B           # 4
QS = 512
NQS = L // QS
KT = L // P
NBH = B * NH_LOC                # 4

F32 = mybir.dt.float32
BF16 = mybir.dt.bfloat16
EXP = mybir.ActivationFunctionType.Exp
NEG = -30000.0


def _build(loop=None):
    nc = bacc.Bacc("TRN2", target_bir_lowering=False, debug=False,
                   num_devices=N_CORES)

    xI = nc.dram_tensor("xI", [P, NBLK, KC, BLK], BF16,
                        kind="ExternalInput").ap()
    wI = nc.dram_tensor("wI", [P, KC, QKV_LOC], BF16,
                        kind="ExternalInput").ap()
    woT = nc.dram_tensor("woT", [D_LOC, H], BF16, kind="ExternalInput").ap()
    cosT = nc.dram_tensor("cosT", [HD, L], BF16, kind="ExternalInput").ap()
    sinTs = nc.dram_tensor("sinTs", [HD, L], BF16, kind="ExternalInput").ap()
    tri = nc.dram_tensor("tri", [P, P], BF16, kind="ExternalInput").ap()
    ones_in = nc.dram_tensor("ones", [P, P], BF16, kind="ExternalInput").ap()
    y = nc.dram_tensor("y", [BL, H], BF16, kind="ExternalOutput").ap()

    with tile.TileContext(nc) as tc, ExitStack() as ctx:
        if loop:
            ctx.enter_context(tc.For_i(0, loop, 1))
        g = ctx.enter_context(tc.tile_pool(name="g", bufs=1))
        wt = g.tile([P, KC, QKV_LOC], BF16)
        cost = g.tile([P, L], BF16)
        sints = g.tile([P, L], BF16)
        k_all = g.tile([P, NBH, L], BF16)
        q_all = g.tile([P, NBH, L], BF16)
        v_all = g.tile([P, NBH, KT, HD], BF16)
        o_all = g.tile([P, B, NH_LOC, L], BF16)
        wo = g.tile([P, NH_LOC, H], BF16)
        trimask = g.tile([P, P], BF16)
        ones = g.tile([P, P], BF16)

        p1x = ctx.enter_context(tc.tile_pool(name="p1x", bufs=2))
        p1q = ctx.enter_context(tc.tile_pool(name="p1q", bufs=2))
        p1t = ctx.enter_context(tc.tile_pool(name="p1t", bufs=4))
        p2e = ctx.enter_context(tc.tile_pool(name="p2e", bufs=8))
        p2t = ctx.enter_context(tc.tile_pool(name="p2t", bufs=6))
        p3y = ctx.enter_context(tc.tile_pool(name="p3y", bufs=3))
        # PSUM: bank-granular. psA: phase-1 qk pairs + phase-3 y groups;
        # psVO: phase-1 v + phase-2 po; psS: score tiles; psD: denominators.
        psA = ctx.enter_context(tc.tile_pool(name="psA", bufs=3, space="PSUM"))
        psVO = ctx.enter_context(tc.tile_pool(name="psVO", bufs=2, space="PSUM"))
        psS = ctx.enter_context(tc.tile_pool(name="psS", bufs=3, space="PSUM"))

        # ------- weight / x(block0) loads, interleaved in kc order -------
        xb0 = p1x.tile([P, KC, BLK], BF16, name="xb")
        for c0_, c1_ in ((0, 1), (1, 2), (2, 4), (4, 8), (8, 12), (12, 16)):
            nc.sync.dma_start(wt[:, c0_:c1_, :], wI[:, c0_:c1_, :])
            nc.sync.dma_start(xb0[:, c0_:c1_, :], xI[:, 0, c0_:c1_, :])
        xb1 = p1x.tile([P, KC, BLK], BF16, name="xb")
        for half in range(2):
            nc.sync.dma_start(
                xb1[:, half * 8:(half + 1) * 8, :],
                xI[:, 1, half * 8:(half + 1) * 8, :])
        for ch in range(4):
            sl = slice(ch * 512, (ch + 1) * 512)
            nc.sync.dma_start(cost[:, sl], cosT[:, sl])
            nc.sync.dma_start(sints[:, sl], sinTs[:, sl])
        nc.sync.dma_start(trimask[:], tri[:])
        nc.sync.dma_start(ones[:], ones_in[:])
        for hh in range(NH_LOC):
            nc.sync.dma_start(wo[:, hh, :], woT[hh * P:(hh + 1) * P, :])

        copy_flip = [0]

        def emit_att(b, qs_i):
            qs = qs_i * QS
            nkt = (qs + QS) // P
            for hh in range(NH_LOC):
                bh = b * NH_LOC + hh
                po = psVO.tile([P, QS], F32, name="vo")
                full = qs // P      # k-tiles with no masking this qs
                if full > 0:
                    esum = p2t.tile([P, QS], BF16, name="esum")
                esum2 = p2t.tile([P, QS], BF16, name="esum")

                # 2-stage software pipeline: emit the score-MM + exp for
                # k+2 before the AV-MM of k, so the exp has ~2 matmuls of
                # latency to complete before the in-order PE needs et(k).
                ets = {}

                def stage1(k_i):
                    d = k_i * P - qs
                    c0 = max(d, 0)
                    psc = psS.tile([P, QS], F32, name="psc")
                    nc.tensor.matmul(
                        psc[:, c0:QS],
                        lhsT=k_all[:, bh, k_i * P:(k_i + 1) * P],
                        rhs=q_all[:, bh, qs + c0:qs + QS],
                        start=True, stop=True)
                    et = p2e.tile([P, QS], BF16, name="et")
                    nc.scalar.activation(et[:, c0:QS], psc[:, c0:QS], EXP)
                    ets[k_i] = et

                def stage2(k_i):
                    d = k_i * P - qs
                    c0 = max(d, 0)
                    et = ets.pop(k_i)
                    if d >= 0:
                        nc.vector.tensor_mul(et[:, d:d + P],
                                             et[:, d:d + P], trimask[:])
                    nc.tensor.matmul(po[:, c0:QS],
                                     lhsT=v_all[:, bh, k_i, :],
                                     rhs=et[:, c0:QS], start=(k_i == 0),
                                     stop=(k_i == nkt - 1))
                    # denominator partials in bf16 on DVE (4x): unmasked
                    # tiles into esum, diagonal ones into esum2, so the pd
                    # PSUM bank is only held for the final MM pair
                    if k_i < full:
                        if k_i == 0:
                            nc.vector.tensor_copy(esum[:], et[:])
                        else:
                            nc.vector.tensor_add(esum[:], esum[:], et[:])
                    elif k_i == full:
                        nc.vector.tensor_copy(esum2[:, c0:QS], et[:, c0:QS])
                    else:
                        nc.vector.tensor_add(esum2[:, c0:QS],
                                             esum2[:, c0:QS], et[:, c0:QS])

                for k_i in range(min(3, nkt)):
                    stage1(k_i)
                for k_i in range(nkt):
                    if k_i + 3 < nkt:
                        stage1(k_i + 3)
                    stage2(k_i)
                pd = psA.tile([P, QS], F32, name="acc")
                nc.tensor.matmul(pd[:], lhsT=ones[:], rhs=esum2[:],
                                 start=True, stop=(full == 0))
                if full > 0:
                    nc.tensor.matmul(pd[:], lhsT=ones[:], rhs=esum[:],
                                     start=False, stop=True)
                rec = p2t.tile([P, QS], F32, name="rec")
                if (b, qs_i) == (1, NQS - 1):
                    # split so the trailing y-projection can pipeline
                    for tq in range(4):
                        s_ = slice(tq * P, (tq + 1) * P)
                        nc.vector.reciprocal(rec[:, s_], pd[:, s_])
                        nc.vector.tensor_mul(
                            o_all[:, b, hh, qs + tq * P:qs + (tq + 1) * P],
                            po[:, s_], rec[:, s_])
                else:
                    nc.vector.reciprocal(rec[:], pd[:])
                    nc.vector.tensor_mul(o_all[:, b, hh, qs:qs + QS],
                                         po[:], rec[:])

        def emit_p3(b, qs_i):
            qs = qs_i * QS
            for tt in range(qs // P, (qs + QS) // P):
                ybig = p3y.tile([P, H], BF16, name="ybig")
                for oc in range(H // QS):
                    py_ = psA.tile([P, QS], F32, name="acc")
                    for hh in range(NH_LOC):
                        nc.tensor.matmul(
                            py_[:],
                            lhsT=o_all[:, b, hh, tt * P:(tt + 1) * P],
                            rhs=wo[:, hh, oc * QS:(oc + 1) * QS],
                            start=(hh == 0), stop=(hh == NH_LOC - 1))
                    if copy_flip[0] % 2 == 0:
                        nc.vector.tensor_copy(
                            ybig[:, oc * QS:(oc + 1) * QS], py_[:])
                    else:
                        nc.scalar.copy(
                            ybig[:, oc * QS:(oc + 1) * QS], py_[:])
                    copy_flip[0] += 1
                rows = slice(b * L + tt * P, b * L + (tt + 1) * P)
                nc.sync.dma_start(y[rows, 0:H // 2], ybig[:, 0:H // 2])
                nc.sync.dma_start(y[rows, H // 2:H], ybig[:, H // 2:H])

        # ---------------- phase 1: QKV projection + RoPE ----------------
        for blk in range(NBLK):
            b, lo = divmod(blk, BLK_PER_B)
            lo *= BLK
            col = blk * BLK
            if blk == 0:
                xb = xb0
            elif blk == 1:
                xb = xb1
            else:
                xb = p1x.tile([P, KC, BLK], BF16, name="xb")
                nc.sync.dma_start(xb[:], xI[:, blk, :, :])

            qc = p1q.tile([P, 4, BLK], BF16, name="qc")
            for dt_i in range(4):     # 0,1 = q heads; 2,3 = k heads
                psum = psA.tile([P, QS], F32, name="acc")
                for kc in range(KC):
                    nc.tensor.matmul(
                        psum[:], lhsT=wt[:, kc, dt_i * P:dt_i * P + P],
                        rhs=xb[:, kc, :],
                        start=(kc == 0), stop=(kc == KC - 1))
                nc.scalar.copy(qc[:, dt_i, :], psum[:])
            # batched rotate-half swap for all 4 dim-tiles
            qsw = p1q.tile([P, 4, BLK], BF16, name="qsw")
            nc.sync.dma_start(qsw[0:64, :, :], qc[64:128, :, :])
            nc.sync.dma_start(qsw[64:128, :, :], qc[0:64, :, :])

            for dt_i in range(4):
                qk, hh = divmod(dt_i, 2)
                bh = b * NH_LOC + hh
                t1 = p1t.tile([P, BLK], BF16, name="t1")
                nc.vector.tensor_mul(t1[:], qc[:, dt_i, :],
                                     cost[:, lo:lo + BLK])
                t2 = p1t.tile([P, BLK], BF16, name="t2")
                nc.vector.tensor_mul(t2[:], qsw[:, dt_i, :],
                                     sints[:, lo:lo + BLK])
                if qk == 0:
                    nc.vector.tensor_add(q_all[:, bh, lo:lo + BLK],
                                         t1[:], t2[:])
                else:
                    nc.vector.tensor_add(k_all[:, bh, lo:lo + BLK],
                                         t1[:], t2[:])

            for tp in range(2):
                psv = psVO.tile([P, QS], F32, name="vo")
                for tj in range(2):
                    tt = tp * 2 + tj
                    for kc in range(KC):
                        nc.tensor.matmul(
                            psv[:, tj * D_LOC:(tj + 1) * D_LOC],
                            lhsT=xb[:, kc, tt * P:(tt + 1) * P],
                            rhs=wt[:, kc, 2 * D_LOC:3 * D_LOC],
                            start=(kc == 0), stop=(kc == KC - 1))
                    kt = lo // P + tt
                    for hh in range(NH_LOC):
                        nc.scalar.copy(
                            v_all[:, b * NH_LOC + hh, kt, :],
                            psv[:, tj * D_LOC + hh * HD:
                                tj * D_LOC + (hh + 1) * HD])

            if blk > 0:
                pb, pq = divmod(blk - 1, BLK_PER_B)
                emit_att(pb, pq)
                emit_p3(pb, pq)

        # ------- final slice: attention + y-projection for (b1, qs3) ----
        emit_att(1, NQS - 1)
        emit_p3(1, NQS - 1)

           full = qs // P      # k-tiles with no masking this qs
                    if full > 0:
                        esum = p2t.tile([P, QS], BF16, name="esum")
                    esum2 = p2t.tile([P, QS], BF16, name="esum")
                    for k_i in range(nkt):
                        d = k_i * P - qs
                        c0 = max(d, 0)
                        psc = psS.tile([P, QS], F32, name="psc")
                        nc.tensor.matmul(
                            psc[:, c0:QS],
                            lhsT=k_all[:, bh, k_i * P:(k_i + 1) * P],
                            rhs=q_all[:, bh, qs + c0:qs + QS],
                            start=True, stop=True)
                        et = p2e.tile([P, QS], BF16, name="et")
                        nc.scalar.activation(et[:, c0:QS], psc[:, c0:QS], EXP)
                        if d >= 0:
                            nc.vector.tensor_mul(et[:, d:d + P],
                                                 et[:, d:d + P], trimask[:])
                        nc.tensor.matmul(po[:, c0:QS],
                                         lhsT=v_all[:, bh, k_i, :],
                                         rhs=et[:, c0:QS], start=(k_i == 0),
                                         stop=(k_i == nkt - 1))
                        # denominator partials in bf16 on DVE (4x mode):
                        # unmasked tiles into esum, diagonal ones into esum2,
                        # so the pd PSUM bank is only held for the final MMs
                        if k_i < full:
                            if k_i == 0:
                                nc.vector.tensor_copy(esum[:], et[:])
                            else:
                                nc.vector.tensor_add(esum[:], esum[:], et[:])
                        elif k_i == full:
                            nc.vector.tensor_copy(esum2[:, c0:QS],
                                                  et[:, c0:QS])
                        else:
                            nc.vector.tensor_add(esum2[:, c0:QS],
                                                 esum2[:, c0:QS],
                                                 et[:, c0:QS])
                    pd = psA.tile([P, QS], F32, name="acc")
                    nc.tensor.matmul(pd[:], lhsT=ones[:], rhs=esum2[:],
                                     start=True, stop=(full == 0))
                    if full > 0:
                        nc.tensor.matmul(pd[:], lhsT=ones[:], rhs=esum[:],
                                         start=False, stop=True)
                    rec = p2t.tile([P, QS], F32, name="rec")
                    if qs_i == NQS - 1:
                        # split so the trailing y-projection can pipeline
                        for tq in range(4):
                            s_ = slice(tq * P, (tq + 1) * P)
                            nc.vector.reciprocal(rec[:, s_], pd[:, s_])
                            nc.vector.tensor_mul(
                                o_all[:, b, hh, qs + tq * P:qs + (tq + 1) * P],
                                po[:, s_], rec[:, s_])
                    else:
                        nc.vector.reciprocal(rec[:], pd[:])
                        nc.vector.tensor_mul(o_all[:, b, hh, qs:qs + QS],
                                             po[:], rec[:])
                # y-projection for this batch's freshly finished qs tokens
                for tt in range(qs // P, (qs + QS) // P):
                    ybig = p3y.tile([P, H], BF16, name="ybig")
                    for oc in range(H // QS):
                        py_ = psA.tile([P, QS], F32, name="acc")
                        for hh in range(NH_LOC):
                            nc.tensor.matmul(
                                py_[:],
                                lhsT=o_all[:, b, hh, tt * P:(tt + 1) * P],
                                rhs=wo[:, hh, oc * QS:(oc + 1) * QS],
                                start=(hh == 0), stop=(hh == NH_LOC - 1))
                        if copy_flip % 2 == 0:
                            nc.vector.tensor_copy(
                                ybig[:, oc * QS:(oc + 1) * QS], py_[:])
                        else:
                            nc.scalar.copy(
                                ybig[:, oc * QS:(oc + 1) * QS], py_[:])
                        copy_flip += 1
                    rows = slice(b * L + tt * P, b * L + (tt + 1) * P)
                    nc.sync.dma_start(y[rows, 0:H // 2], ybig[:, 0:H // 2])
                    nc.sync.dma_start(y[rows, H // 2:H], ybig[:, H // 2:H])

    nc.compile()
    return nc


_NC = None


def _get_nc():
    global _NC
    if _NC is None:
        _NC = _build()
    return _NC


def _host_inputs(x, Wqkv, Wo):
    x = np.asarray(x, dtype=np.float32)
    Wqkv = np.asarray(Wqkv, dtype=np.float32)
    Wo = np.asarray(Wo, dtype=np.float32)

    xTb = x.reshape(BL, H).T.astype(ml_dtypes.bfloat16)   # (H, BL)
    xI = np.ascontiguousarray(
        xTb.reshape(KC, P, NBLK, BLK).transpose(1, 2, 0, 3))

    inv_freq = 1.0 / (ROPE_THETA ** (np.arange(0, HD, 2, dtype=np.float32)
                                     / HD))
    t = np.arange(L, dtype=np.float32)
    freqs = np.outer(t, inv_freq).astype(np.float32)
    emb = np.concatenate([freqs, freqs], axis=-1)
    cosT = np.ascontiguousarray(np.cos(emb).T.astype(ml_dtypes.bfloat16))
    sinT = np.sin(emb).T.astype(np.float32)
    sinTs = np.ascontiguousarray(
        np.concatenate([-sinT[:64], sinT[64:]], 0).astype(ml_dtypes.bfloat16))

    kk = np.arange(P)[:, None]
    qq = np.arange(P)[None, :]
    tri = np.where(qq >= kk, 1.0, 0.0).astype(ml_dtypes.bfloat16)

    scale = np.float32(1.0 / math.sqrt(HD))
    in_maps = []
    for c in range(N_CORES):
        r0 = c * D_LOC
        wq = Wqkv[r0:r0 + D_LOC] * scale
        wk = Wqkv[H + r0:H + r0 + D_LOC]
        wv = Wqkv[2 * H + r0:2 * H + r0 + D_LOC]
        wT_c = np.concatenate([wq, wk, wv], 0).T.astype(ml_dtypes.bfloat16)
        wI_c = np.ascontiguousarray(
            wT_c.reshape(KC, P, QKV_LOC).transpose(1, 0, 2))
        woT_c = np.ascontiguousarray(
            Wo[:, r0:r0 + D_LOC].T.astype(ml_dtypes.bfloat16))
        in_maps.append({
            "xI": xI, "wI": wI_c, "woT": woT_c,
            "cosT": cosT, "sinTs": sinTs, "tri": tri,
            "ones": np.ones((P, P), dtype=ml_dtypes.bfloat16),
        })
    return in_maps


def kernel(x, Wqkv, Wo):
    nc = _get_nc()
    in_maps = _host_inputs(x, Wqkv, Wo)
    res = run_bass_kernel_spmd(nc, in_maps, list(range(N_CORES)))
    y = res.results[0]["y"].astype(np.float64)
    for c in range(1, N_CORES):
        y += res.results[c]["y"].astype(np.float64)
    return y.astype(np.float32).reshape(B, L, H)
